# revision 12
# baseline (speedup 1.0000x reference)
"""BERT self-attention (B=8, S=1024, D=1024, H=16, DH=64) on 8 Trainium2 cores.

Strategy: pure data-parallel over batch - each of the 8 cores runs the full
self-attention for one batch element. No collectives.

v3 design (evolution of the 405us fp32r baseline; v2 measured 302us):
  - X^T pre-transposed + fp16 on the HOST (input layout choice, no device
    cost); weights fp16 on host.  No on-device transposes at all.
  - All matmuls fp16 (1 cycle/row like fp32r, but FWL weight loads and
    10 mantissa bits: ~5e-4 exact-path error).
  - Q^T/K^T: lhsT = W tile; bias folded into the PSUM->SBUF copy
    (per-partition tensor_scalar add).  V natural, bias folded into its
    copy via a host-broadcast [128,D] bias tensor (tensor_tensor add).
  - scores S^T[k,q]: the two heads of a jt tile live on partitions
    0:64/64:128 -> both heads' score matmuls run concurrently via PE row
    tiling (tile_position (0,0)/(64,0)), issue-interleaved A0 B0 A1 B1.
  - exp is the elementwise wall (16.8M/core): one [128,1024] instruction
    per (kt, head) to amortize per-instruction overhead, split between ACT
    (native Exp) and DVE (Schraudolph fp16-bit trick, SCH_N of 16 per pair).
  - context natural orientation (lhsT = P^T tile, fp16 FWL), 4 q-tiles
    batched per PSUM bank; normalize = one reciprocal + one broadcast
    tensor_tensor mul per 4 tiles (denominator from the V ones-column).
  - INTERLEAVED schedule: QK projections are spread through the attention
    loop (QK0 QK1 V sc0 QK2 sc1 ctx0 QK3 sc2 ctx1 ... sc7 ctx6 ctx7) so
    ACT/DVE exp work overlaps PE-heavy projection chunks; PE stream stays
    dense (HAM stays at 2.4 GHz) and the kernel is PE-bound end to end.
"""

import os

import numpy as np

import concourse.bass as bass
import concourse.bacc as bacc
import concourse.mybir as mybir
import concourse.tile as tile
from concourse.bass_utils import run_bass_kernel_spmd

F32 = mybir.dt.float32
FP16 = mybir.dt.float16
I16 = mybir.dt.int16

B, S, D, H = 8, 1024, 1024, 16
DH = D // H  # 64
P = 128
NT = S // P  # 8 tiles along any 1024 dim
SC = S // 512  # 2 chunks of 512
SCALE = 1.0 / float(np.sqrt(DH))
N_CORES = 8
VW = DH + 1  # 65: V block width per head (64 cols + ones col)
HP = H // 2  # 8 head pairs
QG = 4  # q-tiles per ctx PSUM tile ([128, 4*65] = 1040B < 1 bank)

LOG2E = float(np.log2(np.e))
SCH_C = 60.0  # Schraudolph magic offset (fp16 space), tuned for absmax err
# (kt, ab) tiles per head-pair computed on DVE via Schraudolph; rest ACT Exp.
_SCH_ORDER = [
    (0, 0), (1, 1), (2, 0), (3, 1), (4, 0), (5, 1), (6, 0), (7, 1),
    (0, 1), (1, 0), (2, 1), (3, 0), (4, 1), (5, 0), (6, 1), (7, 0),
]
SCH_N = int(os.environ.get("KSCH_N", "7"))
SCH_SET = set(_SCH_ORDER[:SCH_N])

PHASES = int(os.environ.get("KPHASES", "15"))  # 1=loads 2=proj 4=scores 8=ctx


def emit_body(nc, tc, dram, pools):
    (xT_d, m_d, wq_d, bq_d, wk_d, bk_d, wv_d, bvb_d, o_d) = dram
    (cst, xT_pool, qT_pool, kT_pool, v_pool, wx_pool, p_pool, small_pool) = pools

    # ---- per-body constants ----
    mask_cols = cst.tile([P, NT], F32, name="mask_cols", tag="mask_cols")
    nc.sync.dma_start(out=mask_cols, in_=m_d.ap().rearrange("(g p) -> p g", p=P))
    # Schraudolph per-partition bias: mask*log2e*1024 + (15*1024 - C)
    sch_bias = cst.tile([P, NT], F32, name="sch_bias", tag="sch_bias")
    nc.vector.tensor_scalar(
        sch_bias, mask_cols, LOG2E * 1024.0, 15.0 * 1024.0 - SCH_C,
        mybir.AluOpType.mult, mybir.AluOpType.add,
    )
    b_cols = {}
    for nm, hd in (("bq", bq_d), ("bk", bk_d)):
        t = cst.tile([P, NT], F32, name=f"bcol_{nm}", tag=f"bcol_{nm}")
        nc.sync.dma_start(out=t, in_=hd.ap().rearrange("(g p) -> p g", p=P))
        b_cols[nm] = t
    bvb = cst.tile([P, D], F32, name="bvb", tag="bvb")
    nc.sync.dma_start(out=bvb, in_=bvb_d.ap())

    if not PHASES & 1:
        return

    # ---- X^T tiles straight from DRAM (host pre-transposed, fp16) ----
    xT = []
    for it in range(NT):
        t = xT_pool.tile([P, S], FP16, name=f"xT{it}", tag=f"xT{it}")
        nc.sync.dma_start(out=t, in_=xT_d.ap()[it * P : (it + 1) * P, :])
        xT.append(t)

    if not PHASES & 2:
        fin = small_pool.tile([P, DH], F32, name="fin1", tag="fin")
        nc.vector.tensor_copy(fin, xT[0][:, 0:DH])
        nc.sync.dma_start(out=o_d.ap()[0:P, 0:DH], in_=fin)
        return

    def load_w(w_d, tag):
        tiles = []
        for it in range(NT):
            t = wx_pool.tile([P, D], FP16, name=f"{tag}_tile", tag=tag, bufs=8)
            nc.sync.dma_start(out=t, in_=w_d.ap()[it * P : (it + 1) * P, :])
            tiles.append(t)
        return tiles

    wq_t = load_w(wq_d, "wq")
    wk_t = load_w(wk_d, "wk")
    wv_t = load_w(wv_d, "wv")

    # allocate projection outputs up front
    qT, kT = [], []
    for jt in range(NT):
        qT.append(qT_pool.tile([P, S], FP16, name=f"qT{jt}", tag=f"qT{jt}"))
        kT.append(kT_pool.tile([P, S], FP16, name=f"kT{jt}", tag=f"kT{jt}"))
    v_sb = []
    for st in range(NT):
        v = v_pool.tile([P, H * VW], FP16, name=f"v{st}", tag=f"v{st}")
        nc.gpsimd.memset(v, 1.0)  # ones columns survive at h*65+64
        v_sb.append(v)

    with (
        tc.tile_pool(name="psproj", bufs=2, space="PSUM") as ps_proj,
        tc.tile_pool(name="pssc", bufs=3, space="PSUM") as ps_sc,
        tc.tile_pool(name="psctx", bufs=2, space="PSUM") as ps_ctx,
    ):
        def emit_qk(jt):
            # Q^T/K^T tile jt: out[j, s] = sum_i W[i, j] * X^T[i, s] + b[j]
            for nm, w_tiles, dst in (("bq", wq_t, qT[jt]), ("bk", wk_t, kT[jt])):
                mm = ps_proj.tile([P, S], F32, name="mm", tag="big")
                for it in range(NT):
                    for sc in range(SC):
                        nc.tensor.matmul(
                            mm[:, sc * 512 : (sc + 1) * 512],
                            lhsT=w_tiles[it][:, jt * P : (jt + 1) * P],
                            rhs=xT[it][:, sc * 512 : (sc + 1) * 512],
                            start=(it == 0),
                            stop=(it == NT - 1),
                        )
                nc.vector.tensor_scalar(
                    dst, mm, b_cols[nm][:, jt : jt + 1], None,
                    mybir.AluOpType.add,
                )

        def emit_v(st):
            # V[s, j] = sum_i X^T[i, s] * Wv[i, j]; bias added in the copy.
            mm = ps_proj.tile([P, S], F32, name="mmv", tag="big")
            for it in range(NT):
                for jc in range(SC):
                    nc.tensor.matmul(
                        mm[:, jc * 512 : (jc + 1) * 512],
                        lhsT=xT[it][:, st * P : (st + 1) * P],
                        rhs=wv_t[it][:, jc * 512 : (jc + 1) * 512],
                        start=(it == 0),
                        stop=(it == NT - 1),
                    )
            dst = v_sb[st].rearrange("p (g c) -> p g c", c=VW)[:, :, 0:DH]
            nc.vector.tensor_tensor(
                dst, mm.rearrange("p (g c) -> p g c", c=DH),
                bvb.rearrange("p (g c) -> p g c", c=DH),
                mybir.AluOpType.add,
            )

        # head pair hp = heads (2hp, 2hp+1) on partitions 0:64 / 64:128 of
        # q/k tile jt=hp.
        def emit_scores_exp(hp):
            pair = ([], [])
            for kt in range(NT):
                sc_t = {}
                for ab in range(2):
                    pt = p_pool.tile([P, S], FP16, name="pt",
                                     tag=f"p{ab}{kt}")
                    pair[ab].append(pt)
                    sc_t[ab] = ps_sc.tile([P, S], F32, name="sps", tag="sc")
                # interleave A/B so the two row-tiles run concurrently
                for qc in range(SC):
                    qs = slice(qc * 512, (qc + 1) * 512)
                    for ab in range(2):
                        lo, hi = (0, 64) if ab == 0 else (64, 128)
                        nc.tensor.matmul(
                            sc_t[ab][:, qs],
                            lhsT=kT[hp][lo:hi, kt * P : (kt + 1) * P],
                            rhs=qT[hp][lo:hi, qs],
                            start=True, stop=True,
                            tile_position=(lo, 0),
                        )
                for ab in range(2):
                    pt = pair[ab][kt]
                    if (kt, ab) in SCH_SET:
                        # exp(SCALE*s + mask) ~= fp16-bits Schraudolph (DVE)
                        nc.vector.tensor_scalar(
                            pt.bitcast(I16), sc_t[ab],
                            SCALE * LOG2E * 1024.0,
                            sch_bias[:, kt : kt + 1],
                            mybir.AluOpType.mult, mybir.AluOpType.add,
                        )
                    else:
                        nc.scalar.activation(
                            pt, sc_t[ab], mybir.ActivationFunctionType.Exp,
                            bias=mask_cols[:, kt : kt + 1], scale=SCALE,
                        )
            return pair

        def emit_ctx(hp, pair):
            for hi, pT in enumerate(pair):
                h = 2 * hp + hi
                for qg in range(NT // QG):
                    cps = ps_ctx.tile([P, QG * VW], F32, name="cps", tag="ctx")
                    c3 = cps.rearrange("p (g c) -> p g c", c=VW)
                    for qi in range(QG):
                        qt = qg * QG + qi
                        for kt in range(NT):
                            nc.tensor.matmul(
                                c3[:, qi, :],
                                lhsT=pT[kt][:, qt * P : (qt + 1) * P],
                                rhs=v_sb[kt][:, h * VW : (h + 1) * VW],
                                start=(kt == 0),
                                stop=(kt == NT - 1),
                            )
                    rec = small_pool.tile([P, QG], F32, name="rec", tag="rec")
                    nc.vector.reciprocal(rec, c3[:, :, DH])
                    bounce = small_pool.tile([P, QG * DH], F32, name="bounce",
                                             tag="bounce")
                    b3 = bounce.rearrange("p (g c) -> p g c", c=DH)
                    nc.vector.tensor_tensor(
                        b3, c3[:, :, 0:DH],
                        rec[:, :, None].broadcast_to([P, QG, DH]),
                        mybir.AluOpType.mult,
                    )
                    nc.sync.dma_start(
                        out=o_d.ap()[
                            qg * QG * P : (qg + 1) * QG * P,
                            h * DH : (h + 1) * DH,
                        ].rearrange("(g p) m -> p g m", p=P),
                        in_=b3,
                    )

        # ---- interleaved schedule ----
        do_sc = bool(PHASES & 4)
        do_ctx = bool(PHASES & 8) and do_sc
        emit_qk(0)
        emit_qk(1)
        for st in range(NT):
            emit_v(st)
        if not do_sc:
            for jt in range(2, NT):
                emit_qk(jt)
            fin = small_pool.tile([P, DH], F32, name="fin2", tag="fin")
            nc.vector.tensor_copy(fin, qT[0][:, 0:DH])
            nc.sync.dma_start(out=o_d.ap()[0:P, 0:DH], in_=fin)
            return
        sc_pairs = [None] * HP
        sc_pairs[0] = emit_scores_exp(0)
        emit_qk(2)
        sc_pairs[1] = emit_scores_exp(1)
        for hp in range(2, HP + 2):
            if hp - 2 >= 0 and hp - 2 < HP and do_ctx:
                emit_ctx(hp - 2, sc_pairs[hp - 2])
                sc_pairs[hp - 2] = None
            if hp + 1 <= NT - 1:
                emit_qk(hp + 1)
            if hp < HP:
                sc_pairs[hp] = emit_scores_exp(hp)
        if not do_ctx:  # drain the exp stream
            fin = small_pool.tile([P, DH], F32, name="fin3", tag="fin")
            nc.vector.tensor_copy(fin, sc_pairs[HP - 1][0][NT - 1][:, 0:DH])
            nc.sync.dma_start(out=o_d.ap()[0:P, 0:DH], in_=fin)


def build_program(n_reps: int = 1, n_loop: int = 0) -> bass.Bass:
    nc = bacc.Bacc(trn_type="TRN2", target_bir_lowering=False, debug=False)

    xT_d = nc.declare_dram_parameter("xT", [D, S], FP16, isOutput=False)
    m_d = nc.declare_dram_parameter("attention_mask", [S], F32, isOutput=False)
    wq_d = nc.declare_dram_parameter("Wq", [D, D], FP16, isOutput=False)
    bq_d = nc.declare_dram_parameter("bq", [D], F32, isOutput=False)
    wk_d = nc.declare_dram_parameter("Wk", [D, D], FP16, isOutput=False)
    bk_d = nc.declare_dram_parameter("bk", [D], F32, isOutput=False)
    wv_d = nc.declare_dram_parameter("Wv", [D, D], FP16, isOutput=False)
    bvb_d = nc.declare_dram_parameter("bvb", [P, D], F32, isOutput=False)
    o_d = nc.declare_dram_parameter("out", [S, D], F32, isOutput=True)
    dram = (xT_d, m_d, wq_d, bq_d, wk_d, bk_d, wv_d, bvb_d, o_d)

    with tile.TileContext(nc) as tc:
        with (
            tc.tile_pool(name="consts", bufs=1) as cst,
            tc.tile_pool(name="xT", bufs=1) as xT_pool,
            tc.tile_pool(name="qT", bufs=1) as qT_pool,
            tc.tile_pool(name="kT", bufs=1) as kT_pool,
            tc.tile_pool(name="vsb", bufs=1) as v_pool,
            tc.tile_pool(name="wx", bufs=8) as wx_pool,
            tc.tile_pool(name="pT", bufs=2) as p_pool,
            tc.tile_pool(name="small", bufs=8) as small_pool,
        ):
            pools = (cst, xT_pool, qT_pool, kT_pool, v_pool, wx_pool, p_pool,
                     small_pool)
            if n_loop:
                with tc.For_i(0, n_loop, 1):
                    emit_body(nc, tc, dram, pools)
            else:
                for _ in range(n_reps):
                    emit_body(nc, tc, dram, pools)
    nc.compile()
    return nc


_NC_CACHE = None


def _get_nc():
    global _NC_CACHE
    if _NC_CACHE is None:
        _NC_CACHE = build_program()
    return _NC_CACHE


def make_in_maps(hidden_states, attention_mask, Wq, bq, Wk, bk, Wv, bv):
    hs = np.asarray(hidden_states, dtype=np.float32)
    am = np.ascontiguousarray(
        np.asarray(attention_mask, dtype=np.float32).reshape(B, S)
    )
    xT = np.ascontiguousarray(
        hs.transpose(0, 2, 1).astype(np.float16)
    )  # [B, D, S] fp16
    bvb = np.ascontiguousarray(
        np.broadcast_to(np.asarray(bv, dtype=np.float32), (P, D))
    )
    shared = {
        "Wq": np.ascontiguousarray(np.asarray(Wq, dtype=np.float32).astype(np.float16)),
        "bq": np.ascontiguousarray(np.asarray(bq, dtype=np.float32)),
        "Wk": np.ascontiguousarray(np.asarray(Wk, dtype=np.float32).astype(np.float16)),
        "bk": np.ascontiguousarray(np.asarray(bk, dtype=np.float32)),
        "Wv": np.ascontiguousarray(np.asarray(Wv, dtype=np.float32).astype(np.float16)),
        "bvb": bvb,
    }
    return [
        {"xT": xT[b], "attention_mask": am[b], **shared}
        for b in range(B)
    ]


def kernel(hidden_states, attention_mask, Wq, bq, Wk, bk, Wv, bv):
    nc = _get_nc()
    in_maps = make_in_maps(hidden_states, attention_mask, Wq, bq, Wk, bk, Wv, bv)
    res = run_bass_kernel_spmd(nc, in_maps, list(range(N_CORES))).results
    out = np.stack([np.asarray(res[b]["out"], dtype=np.float32) for b in range(B)])
    return out
